# revision 33
# baseline (speedup 1.0000x reference)
"""Trainium2 Bass kernel for nn_Concat_Linear (feat [65536,2,768] -> out [65536,9]).

Data-parallel across 8 NeuronCores (8192 rows each). Per core:

  - feat is staged host-side as fp16 in the exact SBUF tile layout
    [16, 128(f), 12(dj), 512(rows)], so each 512-row chunk loads as ONE fully
    contiguous 1.57 MB DMA on the SP HWDGE ring (128 descriptors x 12 KB).
  - the trilinear form's weights are folded into the projection:
      G[(a,k), b]  = sum_j trans[a,j,k] last[j,b] = (trans . W_int) @ x_last
      TB[(a,k), b] = this[a, b]                   = rep9(W_stim)    @ x_this
    so G and TB come straight out of two K=128 fp16 accumulation chains
    (M=81 and M=105; TB cols 96:105 carry `this` itself for the final linear).
  - epilogue per chunk: m = G*TB (DVE), bil_c = rp@m (PE), LayerNorm stats via
    sq (DVE) + mean-of-squares matmul (PE) + Ln/Exp (ACT) + rstd row-broadcast
    (PE) + scale (DVE), final linear l2@ln + l1@this (PE), bias add (DVE).
    Small PSUM outputs (bil, var, rb, o) co-locate in one PSUM bank at
    32-aligned partition groups via tile_position.
  - emission is wave-pipelined: stage s of chunk c is emitted at wave c+s,
    later stages first, so each engine's FIFO queue is dependency-feasible
    and ~13 chunks are in flight; feat DMAs are emitted 6 chunks ahead.
  - output is stored feature-major ([9, 8192] per core, contiguous 2 KB rows)
    and transposed back to row-major on the host (2.4 MB total -- trivial).
"""

import sys
import types

import numpy as np

B_FULL = 65536
N_CORES = 8
B_CORE = B_FULL // N_CORES
D = 1536       # 2 * 768
NB = 512       # rows per chunk/buffer
NCHUNK = B_CORE // NB
NDJ = 12       # 128-feature blocks
LOOKAHEAD = 6  # buffers of DMA prefetch
LN_EPS = 1e-5


def _ensure_axon_hooks():
    """Register the NTFF profile hook if the image's antenv lacks axon_hooks.

    Without this, trace=True degrades to no profiling (runs still work)."""
    try:
        import antenv  # noqa: F401
        from antenv import axon_hooks  # noqa: F401
        return
    except ImportError:
        pass
    try:
        import antenv
        mod = types.ModuleType("antenv.axon_hooks")
        mod._hook = None
        mod.set_axon_ntff_profile_hook = lambda h: setattr(mod, "_hook", h)
        mod.get_axon_ntff_profile_hook = lambda: mod._hook
        sys.modules["antenv.axon_hooks"] = mod
        antenv.axon_hooks = mod
        from trn_agent_boot.trn_boot import _ntff_profile_via_ctypes
        mod.set_axon_ntff_profile_hook(
            _ntff_profile_via_ctypes("/opt/axon/libaxon_pjrt.so")
        )
    except Exception:
        pass


def make_consts(W_int, W_stim, trans, ln_w, ln_b, W_out, b_out):
    """Host-side constant tensors."""
    W_int = np.asarray(W_int, np.float32)
    W_stim = np.asarray(W_stim, np.float32)
    trans = np.asarray(trans, np.float32)
    ln_w = np.asarray(ln_w, np.float32)
    ln_b = np.asarray(ln_b, np.float32)
    W_out = np.asarray(W_out, np.float32)
    b_out = np.asarray(b_out, np.float32)

    # G[(a,k), b] = sum_j trans[a,j,k] last[j, b], folded through W_int:
    # TW[(a,k), d] = sum_j trans[a,j,k] W_int[j, d]
    TW = np.einsum("ajk,jd->akd", trans, W_int).reshape(81, 768)
    # pad M to 128 cols: FWL (4-xbus fast weight load) needs full-128-col
    # weights; without it each LDWEIGHTS serializes ~110ns against the MMs
    ws_G = np.zeros((128, 6, 128), np.float16)
    ws_G[:, :, 0:81] = TW.reshape(81, 6, 128).transpose(2, 1, 0)

    # TB[(a,k), b] = this[a, b]
    SB = np.repeat(W_stim, 9, axis=0)  # [81, 768]
    ws_T = np.zeros((128, 6, 128), np.float16)
    ws_T[:, :, 0:81] = SB.T.reshape(6, 128, 81).transpose(1, 0, 2)

    # bil_centered[k', b] = sum_a m[a*9+k', b] - (1/9) sum_rows m[row, b]
    rp = np.full((81, 9), -1.0 / 9.0, np.float32)
    for a in range(9):
        for k in range(9):
            rp[a * 9 + k, k] += 1.0

    # Final linear with ln_w/ln_b folded in:
    # out = W_out[:, :9] @ this + (W_out[:, 9:] * ln_w) @ (bil_c * rstd) + b'
    # single final matmul over [TB; pad; ln] (K=105):
    # rows 0:81: TB's 81 rows are 9 identical copies of this[a]; contracting
    # with W_out/9 gives l1 @ this without materializing `this`.
    # rows 96:105: l2 (ln_w folded) against the LN rows written into t_sb
    # (at partition 96 -- DVE writes need a 32-aligned base partition).
    l290 = np.zeros((105, 9), np.float32)
    l290[0:81, :] = np.repeat(W_out[:, 0:9].T / 9.0, 9, axis=0)
    l290[96:105, :] = (W_out[:, 9:18] * ln_w[None, :]).T
    bout = (b_out + W_out[:, 9:18] @ ln_b).reshape(9, 1).astype(np.float32)

    f16 = np.float16
    return {
        "ws_G": ws_G, "ws_T": ws_T, "rp": rp.astype(f16),
        "l290": l290.astype(f16), "bout": bout,
        "o99": np.full((9, 1), 1.0 / 9.0, f16),
        "o19": np.ones((1, 9), np.float32),  # f32: pairs with f32 rstd
        "eps": np.full((1, 1), LN_EPS, np.float32),
    }


def build_program(b_core=B_CORE, num_devices=N_CORES):
    import concourse.bass as bass  # noqa: F401
    import concourse.tile as tile
    from concourse import bacc, mybir

    f32 = mybir.dt.float32
    f16 = mybir.dt.float16
    nc = bacc.Bacc("TRN2", target_bir_lowering=False, debug=False,
                   num_devices=num_devices)

    feat_d = nc.dram_tensor("feat", [NCHUNK, 128, NDJ, NB], f16,
                            kind="ExternalInput")
    out_d = nc.dram_tensor("out", [9, b_core], f32, kind="ExternalOutput")
    cshapes = {
        "ws_G": [128, 6, 128], "ws_T": [128, 6, 128], "rp": [81, 9],
        "l290": [105, 9], "bout": [9, 1],
        "o99": [9, 1], "o19": [1, 9], "eps": [1, 1],
    }
    f32_keys = {"bout", "eps", "o19"}

    def cdt(k):
        return f32 if k in f32_keys else f16
    cd = {k: nc.dram_tensor(k, v, cdt(k), kind="ExternalInput")
          for k, v in cshapes.items()}

    with tile.TileContext(nc) as tc:
        with tc.tile_pool(name="consts", bufs=1) as cp, \
             tc.tile_pool(name="tr", bufs=LOOKAHEAD + 2) as trp, \
             tc.tile_pool(name="gsb", bufs=4) as gsbp, \
             tc.tile_pool(name="tsb", bufs=12) as tsbp, \
             tc.tile_pool(name="msb", bufs=4) as msbp, \
             tc.tile_pool(name="bsb", bufs=8) as bsbp, \
             tc.tile_pool(name="ssb", bufs=4) as ssbp, \
             tc.tile_pool(name="osb", bufs=6) as osbp, \
             tc.tile_pool(name="gps", bufs=2, space="PSUM") as gpp, \
             tc.tile_pool(name="tps", bufs=2, space="PSUM") as tpp, \
             tc.tile_pool(name="bilps", bufs=2, space="PSUM") as blp, \
             tc.tile_pool(name="varps", bufs=1, space="PSUM") as vrp, \
             tc.tile_pool(name="ops", bufs=1, space="PSUM") as opp, \
             tc.tile_pool(name="rbsb", bufs=4) as rbsbp:

            cs = {k: cp.tile(v, cdt(k), tag=k, name=k)
                  for k, v in cshapes.items()}
            for k in cshapes:
                # consts load on the ACT HWDGE queue so the SP queue can
                # start streaming feat immediately
                nc.scalar.dma_start(cs[k][:], cd[k].ap())

            st = {}  # per-chunk live tiles

            def s_load(c):
                # split G-half / TB-half so the G matmuls can start after
                # 0.8 MB instead of 1.57 MB (shorter pipeline ramp)
                trt = trp.tile([128, NDJ, NB], f16, tag="tr", name=f"tr{c}")
                nc.sync.dma_start(trt[:, 0:6, :], feat_d.ap()[c][:, 0:6, :])
                nc.sync.dma_start(trt[:, 6:12, :], feat_d.ap()[c][:, 6:12, :])
                st[c] = {"tr": trt}

            def s0_proj(c):
                # paired across (c, c+1): each dj weight is loaded once and
                # both chunks' matmuls run back-to-back on it, so the second
                # MM pipelines (~216ns) instead of paying LDW + full latency
                if c % 2 == 1:
                    return
                cc = [c] + ([c + 1] if c + 1 < NCHUNK else [])
                for ci in cc:
                    st[ci]["g_ps"] = gpp.tile([128, NB], f32, tag="g",
                                              name=f"g{ci}")
                    st[ci]["t_ps"] = tpp.tile([128, NB], f32, tag="t",
                                              name=f"t{ci}")
                for dj in range(6):
                    for ci in cc:
                        nc.tensor.matmul(st[ci]["g_ps"][:],
                                         cs["ws_G"][:, dj, :],
                                         st[ci]["tr"][:, dj, :],
                                         start=(dj == 0), stop=(dj == 5))
                for dj in range(6, 12):
                    for ci in cc:
                        nc.tensor.matmul(st[ci]["t_ps"][:],
                                         cs["ws_T"][:, dj - 6, :],
                                         st[ci]["tr"][:, dj, :],
                                         start=(dj == 6), stop=(dj == 11))

            def s1_gcopy(c):
                # on DVE: an ACT copy would thrash activation-table sets
                # against Sqrt (~1.3us per ACT_TABLE_LOAD)
                t = st[c]
                g_sb = gsbp.tile([81, NB], f16, tag="gsb", name=f"gsb{c}")
                nc.vector.tensor_copy(g_sb[:], t["g_ps"][0:81, :])
                t["g_sb"] = g_sb

            def s2_tcopy(c):
                # [105, NB]: rows 0:81 = TB copy; rows 96:105 written later by
                # s11_lnmul so s12 contracts [TB; ln] in one K=105 matmul
                t = st[c]
                t_sb = tsbp.tile([105, NB], f16, tag="tsb", name=f"tsb{c}")
                # rows 81:96 come from ws_T's zero padding -> zeros for free
                # (l290 is zero there; avoids uninitialized-SBUF reads)
                nc.vector.tensor_copy(t_sb[0:96, :], t["t_ps"][0:96, :])
                t["t_sb"] = t_sb

            def s3_m(c):
                t = st[c]
                m_sb = msbp.tile([81, NB], f16, tag="msb", name=f"msb{c}")
                nc.vector.tensor_mul(m_sb[:], t["g_sb"][:], t["t_sb"][0:81, :])
                t["m_sb"] = m_sb

            def s4_bil(c):
                t = st[c]
                bil_ps = blp.tile([9, NB], f32, tag="bil", name=f"bil{c}")
                nc.tensor.matmul(bil_ps[:], cs["rp"][:], t["m_sb"][:],
                                 start=True, stop=True)
                t["bil_ps"] = bil_ps

            def s5_bilcopy(c):
                t = st[c]
                bil_sb = bsbp.tile([9, NB], f16, tag="bsb", name=f"bsb{c}")
                nc.vector.tensor_copy(bil_sb[:], t["bil_ps"][:])
                t["bil_sb"] = bil_sb

            def s6_sq(c):
                t = st[c]
                sq_sb = ssbp.tile([9, NB], f16, tag="ssb", name=f"ssb{c}")
                nc.vector.tensor_mul(sq_sb[:], t["bil_sb"][:], t["bil_sb"][:])
                t["sq_sb"] = sq_sb

            def s7_var(c):
                t = st[c]
                # o99 has 1/9 folded in -> var = mean_k bil_c^2
                var_ps = vrp.tile([1, NB], f32, tag="var", name=f"var{c}")
                nc.tensor.matmul(var_ps[:], cs["o99"][:], t["sq_sb"][:],
                                 start=True, stop=True)
                t["var_ps"] = var_ps

            def s8_std(c):
                # Sqrt + Identity both live in the `sqrt_and_others` ACT
                # table set -> one ACT_TABLE_LOAD for the whole kernel
                # (Ln+Exp here would alternate two sets, ~2.6us per chunk)
                t = st[c]
                std = ssbp.tile([1, NB], f32, tag="lnv", name=f"std{c}")
                nc.scalar.activation(std[:], t["var_ps"][:],
                                     mybir.ActivationFunctionType.Sqrt,
                                     bias=cs["eps"][:, 0:1])
                t["std"] = std

            def s9_rstd(c):
                t = st[c]
                rstd = ssbp.tile([1, NB], f32, tag="rstd", name=f"rstd{c}")
                nc.vector.reciprocal_approx_fast(rstd[:], t["std"][:])
                t["rstd"] = rstd

            def s10_rb(c):
                # row-broadcast on the (idle) GPSIMD engine: frees a PE
                # matmul + LDWEIGHTS + a PSUM bank per chunk
                t = st[c]
                rb_sb = rbsbp.tile([9, NB], f32, tag="rbsb", name=f"rb{c}")
                nc.gpsimd.partition_broadcast(rb_sb[:], t["rstd"][:])
                t["rb_sb"] = rb_sb

            def s11_lnmul(c):
                # writes the LN rows straight into t_sb[96:105] (32-aligned base)
                t = st[c]
                nc.vector.tensor_mul(t["t_sb"][96:105, :], t["rb_sb"][:],
                                     t["bil_sb"][:])

            def s12_o(c):
                t = st[c]
                o_ps = opp.tile([9, NB], f32, tag="o", name=f"o{c}")
                nc.tensor.matmul(o_ps[:], cs["l290"][:], t["t_sb"][:],
                                 start=True, stop=True)
                t["o_ps"] = o_ps

            def s13_add(c):
                # Identity(+bias) lives in every ACT table set -> no reload
                t = st[c]
                osb = osbp.tile([9, NB], f32, tag="osb", name=f"osb{c}")
                nc.scalar.activation(osb[:], t["o_ps"][:],
                                     mybir.ActivationFunctionType.Identity,
                                     bias=cs["bout"][:, 0:1])
                t["osb"] = osb

            def s14_store(c):
                t = st[c]
                nc.sync.dma_start(out_d.ap()[:, c * NB:(c + 1) * NB],
                                 t["osb"][:])

            stages = [s0_proj, s1_gcopy, s2_tcopy, s3_m, s4_bil, s5_bilcopy,
                      s6_sq, s7_var, s8_std, s9_rstd, s10_rb, s11_lnmul,
                      s12_o, s13_add, s14_store]
            S = len(stages)

            for c in range(min(LOOKAHEAD, NCHUNK)):
                s_load(c)
            for w in range(NCHUNK + S - 1):
                lc = w + LOOKAHEAD
                if lc < NCHUNK:
                    s_load(lc)
                for s in reversed(range(S)):
                    c = w - s
                    if 0 <= c < NCHUNK:
                        stages[s](c)
    nc.compile()
    return nc


_PROGRAM = None


def _get_program():
    global _PROGRAM
    if _PROGRAM is None:
        _PROGRAM = build_program()
    return _PROGRAM


def kernel(feat, W_int, W_stim, trans, ln_w, ln_b, W_out, b_out,
           trace=False, trace_kwargs=None):
    _ensure_axon_hooks()
    from concourse.bass_utils import run_bass_kernel_spmd

    feat = np.asarray(feat, np.float32)
    # stage[c, ib, f, dj, r] = feat16[c*B_CORE + ib*NB + r, 128*dj + f]
    feat2 = feat.reshape(N_CORES, NCHUNK, NB, NDJ, 128).astype(np.float16)
    stage = np.ascontiguousarray(feat2.transpose(0, 1, 4, 3, 2))
    consts = make_consts(W_int, W_stim, trans, ln_w, ln_b, W_out, b_out)
    nc = _get_program()
    in_maps = []
    for c in range(N_CORES):
        m = {"feat": stage[c]}
        m.update(consts)
        in_maps.append(m)
    res = run_bass_kernel_spmd(nc, in_maps, list(range(N_CORES)), trace=trace)
    out = np.concatenate(
        [res.results[c]["out"].T for c in range(N_CORES)], axis=0)
    kernel.last_results = res
    return np.ascontiguousarray(out, dtype=np.float32)
